# revision 1
# baseline (speedup 1.0000x reference)
"""Trainium2 Bass kernel for nn_CONV_tiny_add_partial_558345748883.

Network: 3x [conv5x5(pad2) -> BN -> avgpool2 -> clip01] -> conv4x4(valid) -> BN1d
Input x_in [1024, 3, 32, 32] f32; output [1024, 10] f32.  ~99 us on 8 cores.

Strategy
--------
- Data parallel: batch 1024 split over 8 NeuronCores (128 samples each).
- Each conv+BN+pool block is algebraically folded into one stride-2 6x6 conv
  (pool/BN are linear: pool(bn(conv(x))) == stride2conv(x; W') + beta), which
  cuts PE work ~2.8x and removes all pooling vector work. BN scale folds into
  the conv weights.
- The TensorEngine here is column-rate-bound, not MAC-bound (a full 128x128
  matmul and four concurrent 32x32 sub-tile matmuls cost the same per moving
  column, ~2GHz sustained). So convs run as EIGHT concurrent PE sub-tiles
  (tile_position 4 rows x 2 cols), one matmul per kernel tap, each tile
  streaming a different sample block; all taps of a block accumulate into one
  PSUM bank region (per-partition has_written semantics allow several col
  tiles per bank).
- L1's contraction is packed to K=18 = (6 dy) x (3 ci) with dy shifts and
  stride-2 row decimation baked into a host-side im2row layout (6 dx taps).
- PSUM eviction applies Relu(x + beta) with per-partition bias, split between
  ScalarE and VectorE (both support partition-shifted writes), scattering
  straight into the next layer's zero-bordered padded windows; the upper
  clip is one strided tensor_scalar_min per supergroup.
- float16 operands (fp32 PSUM accumulate): rel err ~3e-4.
- One rotating 8-slot PSUM pool; all activations stay resident in SBUF; the
  only DRAM traffic is the im2row input (5.3 MB/core) and the 10x128 output.
"""
import os
import sys
import numpy as np

for _p in ("/opt/trn_rl_repo", "/root/.axon_site/_ro/trn_rl_repo"):
    if os.path.isdir(_p) and _p not in sys.path:
        sys.path.append(_p)

import concourse.bass as bass
import concourse.bacc as bacc
import concourse.mybir as mybir
from concourse.tile import TileContext

EPS = 1e-5
N_CORES = 8
DT = mybir.dt.float16
F32 = mybir.dt.float32
AF = mybir.ActivationFunctionType

# sizes (mutable via configure() for small-scale simulation tests)
NW = 2    # waves per core
Q = 16    # samples per lane per wave
S = NW * 4 * Q          # samples per core
HQ = Q // 2             # samples per lane per half-wave (L1 dma granularity)
NQ = NW * Q             # per-lane total samples
USE_CLEARS = False  # True: belt-and-braces PSUM bank clear matmuls (needed for CoreSim's
                    # bank-granular accumulation-group model; HW has_written is per-partition)


def configure(nw, q, use_clears=None):
    global NW, Q, S, HQ, NQ, _NC_CACHE, USE_CLEARS
    if use_clears is not None:
        USE_CLEARS = use_clears
    NW, Q = nw, q
    S = NW * 4 * Q
    HQ = Q // 2
    NQ = NW * Q
    _NC_CACHE = None


# ----------------------------------------------------------------------------
# Host-side prep
# ----------------------------------------------------------------------------

def _fold_w(w, g, b, m, v):
    """Fold conv5x5 + BN + avgpool2 into stride-2 6x6 weights + bias."""
    inv = g / np.sqrt(v + EPS)
    Wp = np.zeros((w.shape[0], w.shape[1], 6, 6), np.float32)
    for r in (0, 1):
        for s_ in (0, 1):
            Wp[:, :, r:r + 5, s_:s_ + 5] += w
    Wp *= 0.25 * inv[:, None, None, None]
    beta = (b - m * inv).astype(np.float32)
    return Wp.astype(np.float32), beta


def _lane_rep(a, groups=4):
    """Replicate [p, f] into [128, f] across partition groups of 32."""
    out = np.zeros((128, a.shape[1]), np.float32)
    for j in range(groups):
        out[32 * j:32 * j + a.shape[0]] = a
    return out


def host_prep_weights(inputs):
    W1, beta1 = _fold_w(inputs['w1'], inputs['g1'], inputs['b1'], inputs['m1'], inputs['v1'])
    W2, beta2 = _fold_w(inputs['w2'], inputs['g2'], inputs['b2'], inputs['m2'], inputs['v2'])
    W3, beta3 = _fold_w(inputs['w3'], inputs['g3'], inputs['b3'], inputs['m3'], inputs['v3'])
    inv4 = inputs['g4'] / np.sqrt(inputs['v4'] + EPS)
    beta4 = (inputs['b4'] - inputs['m4'] * inv4).astype(np.float32)
    W4 = (inputs['w4'] * inv4[:, None, None, None]).astype(np.float32)

    d = {}
    # L1 lhsT per dx tap f: wl1[dy*3+ci, f*32+co] = W1[co, ci, dy, f]
    d['wl1'] = _lane_rep(W1.transpose(2, 1, 3, 0).reshape(18, 6 * 32)).astype(np.float16)
    # L2 lhsT per tap t=e*6+f: [32 ci, 32 co]
    d['wl2'] = _lane_rep(W2.transpose(1, 2, 3, 0).reshape(32, 36 * 32)).astype(np.float16).astype(np.float16)
    # L3 lhsT per tap: [32 ci, 64 co]
    d['wl3'] = _lane_rep(W3.transpose(1, 2, 3, 0).reshape(32, 36 * 64)).astype(np.float16)
    # L4 lhsT per tap t=u*4+v: [64 ci, 10 co], replicated into both row halves
    wl4 = W4.transpose(1, 2, 3, 0).reshape(64, 16 * 10)
    out4 = np.zeros((128, 160), np.float32)
    out4[0:64] = wl4
    out4[64:128] = wl4
    d['wl4'] = out4.astype(np.float16)

    bt = np.zeros((128, 5), np.float32)
    bt[:, 0] = np.tile(beta1, 4)
    bt[:, 1] = np.tile(beta2, 4)
    bt[:, 2] = np.tile(beta3, 2)
    bt[0:10, 3] = beta4
    bt[0:64, 4] = 1.0 - np.tile(beta2, 2)
    d['betas'] = bt
    return d


def host_prep_x(x_core):
    """[S, 3, 32, 32] -> x_l1 [4, 18, NQ, 576] im2row layout.

    x_l1[lane, dy*3+ci, qg, r*36+c] = xpad[4*qg+lane, ci, 2r+dy, c]
    """
    Sc = x_core.shape[0]
    xp = np.zeros((Sc, 3, 36, 36), np.float32)
    xp[:, :, 2:34, 2:34] = x_core
    arr = np.stack([xp[:, :, dy:dy + 32:2, :] for dy in range(6)], axis=1)  # [S,6,3,16,36]
    arr = arr.reshape(Sc, 18, 16 * 36)
    x_l1 = arr.reshape(Sc // 4, 4, 18, 576).transpose(1, 2, 0, 3)
    return np.ascontiguousarray(x_l1).astype(np.float16)


# ----------------------------------------------------------------------------
# Bass program
# ----------------------------------------------------------------------------

def build_program():
    nc = bacc.Bacc(target_bir_lowering=False)

    x_l1 = nc.dram_tensor("x_l1", [4, 18, NQ, 576], DT, kind="ExternalInput")
    wl1 = nc.dram_tensor("wl1", [128, 192], DT, kind="ExternalInput")
    wl2 = nc.dram_tensor("wl2", [128, 1152], DT, kind="ExternalInput")
    wl3 = nc.dram_tensor("wl3", [128, 2304], DT, kind="ExternalInput")
    wl4 = nc.dram_tensor("wl4", [128, 160], DT, kind="ExternalInput")
    betas = nc.dram_tensor("betas", [128, 5], F32, kind="ExternalInput")
    y = nc.dram_tensor("y", [10, 4 * NQ], F32, kind="ExternalOutput")

    TAPS = [(e, f) for e in range(6) for f in range(6)]

    with TileContext(nc) as tc:
        with tc.tile_pool(name="consts", bufs=1) as cpool:
            # ---- constants ----
            wl1_t = cpool.tile([128, 192], DT, name="wl1_t")
            wl2_t = cpool.tile([128, 1152], DT, name="wl2_t")
            wl3_t = cpool.tile([128, 2304], DT, name="wl3_t")
            wl4_t = cpool.tile([128, 160], DT, name="wl4_t")
            betas_t = cpool.tile([128, 5], F32, name="betas_t")
            nc.sync.dma_start(wl1_t[:, :], wl1.ap())
            nc.sync.dma_start(betas_t[:, :], betas.ap())
            deferred_w = []  # issued after the first input DMAs

            def flush_weights():
                if not deferred_w:
                    return
                nc.scalar.dma_start(wl2_t[:, :], wl2.ap())
                nc.scalar.dma_start(wl3_t[:, :], wl3.ap())
                nc.scalar.dma_start(wl4_t[:, :], wl4.ap())
                deferred_w.clear()
            deferred_w.append(1)

            # ---- persistent activation tiles ----
            l2in = [cpool.tile([128, Q * 400], DT, name=f"l2in{i}") for i in range(2)]
            l3in = [cpool.tile([128, Q * 144], DT, name=f"l3in{i}") for i in range(2)]
            stagA = cpool.tile([128, NQ * 16], DT, name="stagA")
            stagB = cpool.tile([128, NQ * 16], DT, name="stagB")
            out_sb = cpool.tile([128, 4 * NQ], F32, name="out_sb")

            def memset_borders(t_, W_, nrows):
                # zero only the pad borders of each sample window (2-wide)
                v = t_.rearrange("p (s v) -> p s v", v=W_ * nrows)
                # top 2 + bottom 2 rows (contiguous 2*W each)
                nc.vector.memset(
                    bass.AP(v.tensor, v.offset,
                            [v.ap[0], v.ap[1], [(nrows - 2) * W_, 2], [1, 2 * W_]]),
                    0.0)
                # left 2 + right 2 cols of middle rows
                nc.vector.memset(
                    bass.AP(v.tensor, v.offset + 2 * W_,
                            [v.ap[0], v.ap[1], [W_, nrows - 4], [W_ - 2, 2], [1, 2]]),
                    0.0)

            for t_ in (l2in[0], l3in[0], l2in[1], l3in[1]):
                W_, nr = (20, 20) if t_.tensor.shape[1] == Q * 400 else (12, 12)
                memset_borders(t_, W_, nr)

            def evac(engine_is_act, dst, src_ap, bias_ap):
                """max(x + beta, 0) on ACT or DVE with partition shift."""
                if engine_is_act:
                    nc.scalar.activation(dst, src_ap, AF.Relu,
                                         bias=bias_ap, scale=1.0)
                else:
                    nc.vector.tensor_scalar(
                        dst, src_ap, bias_ap, 0.0,
                        mybir.AluOpType.add, mybir.AluOpType.max)

            with (
                tc.tile_pool(name="l1io", bufs=4) as l1pool,
                tc.tile_pool(name="ps", bufs=8, space="PSUM") as pspool,
            ):
                for w in range(NW):
                    l2t, l3t = l2in[w % 2], l3in[w % 2]
                    # ================= L1 =================
                    # 8 concurrent tiles (4 rows x 2 cols); tile (r, c) does
                    # lane-r samples q = 4*sg + 2*c + {0,1}; bank per row r.
                    NSG = Q // 4
                    for sg in range(NSG):
                        if sg % 2 == 0:
                            nsgl = min(2, NSG - sg) * 4
                            l1t2 = l1pool.tile([128, 8 * 576], DT, name="l1t", tag="l1t")
                            l1d = l1t2.rearrange("p (s v) -> p s v", v=576)
                            q0 = w * Q + 4 * sg
                            for r in range(4):
                                eng = nc.sync if r % 2 == 0 else nc.scalar
                                eng.dma_start(
                                    l1d[32 * r:32 * r + 18, 0:nsgl, :],
                                    x_l1.ap()[r, :, q0:q0 + nsgl, :],
                                )
                        l1v = l1t2.rearrange("p (s rr cc) -> p s rr cc", s=8, rr=16)
                        so = 4 * (sg % 2)
                        if sg == NSG - 1 or NSG == 1:
                            flush_weights()
                        pl1 = [pspool.tile([128, 512], F32, name=f"ps1_{r}", tag="ps")
                               for r in range(4)]
                        for f in range(6):
                            for r in range(4):
                                lhsT = wl1_t[32 * r:32 * r + 18, 32 * f:32 * f + 32]
                                for c in range(2):
                                    rhs = l1v[32 * r:32 * r + 18, so + 2 * c:so + 2 * c + 2,
                                              :, f:f + 31:2]
                                    nc.tensor.matmul(
                                        pl1[r][32 * c:32 * c + 32, :], lhsT, rhs,
                                        start=(f == 0), stop=False,
                                        skip_group_check=True,
                                        tile_position=(32 * r, 32 * c),
                                    )
                        # evac: rows 0,1 on ACT; rows 2,3 on DVE
                        l2vv = l2t.rearrange("p (s yy xx) -> p s yy xx", s=Q, yy=20)
                        for r in range(4):
                            gb = 64 * (r % 2)
                            slot0 = 4 * sg + 2 * (r // 2)
                            src_ap = pl1[r][0:64, :].rearrange(
                                "p (k yy xx) -> p k yy xx", k=2, yy=16)
                            dst = l2vv[gb:gb + 64, slot0:slot0 + 2, 2:18, 2:18]
                            evac(r < 2, dst, src_ap, betas_t[gb:gb + 64, 0:1])
                        nc.vector.tensor_scalar_min(
                            l2t[:, 4 * sg * 400:4 * (sg + 1) * 400],
                            l2t[:, 4 * sg * 400:4 * (sg + 1) * 400], 1.0)

                    # ================= L2 =================
                    # 8 concurrent tiles: row r2 = l2in group, col c2 = which
                    # half of its 16 samples; bank per row r2.
                    l2v = l2t.rearrange("p (s yy xx) -> p s yy xx", s=Q, yy=20)
                    pl2 = [pspool.tile([128, 512], F32, name=f"ps2_{r}", tag="ps")
                           for r in range(4)]
                    H2 = Q // 2
                    for t, (e, f) in enumerate(TAPS):
                        for r2 in range(4):
                            lhsT = wl2_t[32 * r2:32 * r2 + 32, 32 * t:32 * t + 32]
                            for c2 in range(2):
                                rhs = l2v[32 * r2:32 * r2 + 32,
                                          H2 * c2:H2 * (c2 + 1), e:e + 15:2, f:f + 15:2]
                                nc.tensor.matmul(
                                    pl2[r2][32 * c2:32 * c2 + 32, 0:H2 * 64], lhsT, rhs,
                                    start=(t == 0), stop=False,
                                    skip_group_check=True,
                                    tile_position=(32 * r2, 32 * c2),
                                )
                    l3vv = l3t.rearrange("p (s yy xx) -> p s yy xx", s=Q, yy=12)
                    for r2 in range(4):
                        gb = 64 * (r2 % 2)
                        slot0 = H2 * (r2 // 2)
                        src_ap = pl2[r2][0:64, 0:H2 * 64].rearrange(
                            "p (k yy xx) -> p k yy xx", k=H2, yy=8)
                        dst = l3vv[gb:gb + 64, slot0:slot0 + H2, 2:10, 2:10]
                        evac(r2 < 2, dst, src_ap, betas_t[gb:gb + 64, 1:2])
                    nc.vector.tensor_scalar_min(l3t[:, :], l3t[:, :], 1.0)

                    # ================= L3 =================
                    l3v = l3t.rearrange("p (s yy xx) -> p s yy xx", s=Q, yy=12)
                    pl3 = [pspool.tile([128, 256], F32, name=f"ps3_{r}", tag="ps")
                           for r in range(4)]
                    for t, (e, f) in enumerate(TAPS):
                        for r3 in range(4):
                            c3 = r3 // 2
                            lhsT = wl3_t[32 * r3:32 * r3 + 32, 64 * t:64 * t + 64]
                            rhs = l3v[32 * r3:32 * r3 + 32, :, e:e + 7:2, f:f + 7:2]
                            nc.tensor.matmul(
                                pl3[r3][64 * c3:64 * c3 + 64, 0:Q * 16], lhsT, rhs,
                                start=(t == 0), stop=(t == 35),
                                skip_group_check=True,
                                tile_position=(32 * r3, 64 * c3),
                            )
                    for r3 in range(4):
                        c3 = r3 // 2
                        stag = stagA if r3 % 2 == 0 else stagB
                        nc.scalar.activation(
                            stag[64 * c3:64 * c3 + 64, w * Q * 16:(w + 1) * Q * 16],
                            pl3[r3][64 * c3:64 * c3 + 64, 0:Q * 16],
                            AF.Relu, bias=betas_t[64 * c3:64 * c3 + 64, 2:3], scale=1.0,
                        )
                    for stag in (stagA, stagB):
                        nc.vector.tensor_scalar_min(
                            stag[:, w * Q * 16:(w + 1) * Q * 16],
                            stag[:, w * Q * 16:(w + 1) * Q * 16], 1.0)

                # ================= L4 =================
                streams = [(stagA, 0), (stagA, 1), (stagB, 0), (stagB, 1)]
                ps4s = [pspool.tile([128, NQ], F32, name=f"ps4_{k}", tag="ps")
                        for k in range(4)]
                for t in range(16):
                    for k, (stag, r) in enumerate(streams):
                        sv = stag.rearrange("p (n t) -> p n t", t=16)
                        lhsT = wl4_t[64 * r:64 * r + 64, 10 * t:10 * t + 10]
                        rhs = sv[64 * r:64 * r + 64, :, t]
                        nc.tensor.matmul(
                            ps4s[k][0:10, :], lhsT, rhs,
                            start=(t == 0), stop=(t == 15),
                            skip_group_check=True,
                            tile_position=(64 * r, 0),
                        )
                for k in range(4):
                    nc.scalar.activation(
                        out_sb[0:10, k * NQ:(k + 1) * NQ], ps4s[k][0:10, :],
                        AF.Identity, bias=betas_t[0:10, 3:4], scale=1.0,
                    )
                nc.sync.dma_start(y.ap(), out_sb[0:10, :])

        return nc


_NC_CACHE = None


def get_program():
    global _NC_CACHE
    if _NC_CACHE is None:
        nc = build_program()
        if not nc.is_finalized():
            nc.finalize()
        _NC_CACHE = nc
    return _NC_CACHE


def make_in_maps(inputs, n_cores=N_CORES):
    wdict = host_prep_weights(inputs)
    in_maps = []
    for c in range(n_cores):
        x_core = np.asarray(inputs['x_in'][c * S:(c + 1) * S], np.float32)
        m = {'x_l1': host_prep_x(x_core)}
        m.update(wdict)
        in_maps.append(m)
    return in_maps


def _phys_sample(k, n):
    """Output stream k, slot n -> per-core sample index (v5 8-tile mapping)."""
    H2 = Q // 2
    w, m = divmod(n, Q)
    r3 = [0, 2, 1, 3][k]
    g3 = r3
    r2 = 2 * (m // H2) + (g3 // 2)
    n1 = H2 * (g3 % 2) + (m % H2)
    g = r2
    r1 = 2 * ((n1 % 4) // 2) + (g // 2)
    q = 4 * (n1 // 4) + 2 * (g % 2) + (n1 % 2)
    return 4 * Q * w + 4 * q + r1


def assemble_output(results, n_cores=N_CORES):
    """results: list of per-core dicts with y [10, 4*NQ] -> [n_cores*S, 10]."""
    out = np.zeros((n_cores * S, 10), np.float32)
    for c in range(n_cores):
        yc = np.asarray(results[c]['y'])  # [10, 4*NQ]
        for k in range(4):
            for n in range(NQ):
                out[c * S + _phys_sample(k, n), :] = yc[:, k * NQ + n]
    return out


def kernel(**inputs) -> np.ndarray:
    from concourse.bass_utils import run_bass_kernel_spmd
    nc = get_program()
    in_maps = make_in_maps(inputs)
    res = run_bass_kernel_spmd(nc, in_maps, list(range(N_CORES)))
    return assemble_output(res.results)



# revision 2
# speedup vs baseline: 1.1041x; 1.1041x over previous
"""Trainium2 Bass kernel for nn_CONV_tiny_add_partial_558345748883.

Network: 3x [conv5x5(pad2) -> BN -> avgpool2 -> clip01] -> conv4x4(valid) -> BN1d
Input x_in [1024, 3, 32, 32] f32; output [1024, 10] f32.

v2 strategy
-----------
- Data parallel: batch 1024 over 8 NeuronCores (128 samples each), one wave.
- Each conv+BN+pool block folded into one stride-2 6x6 conv (+bias), fp16.
- All activation layouts are SAMPLE-INNERMOST ([part, y, x, s]) so every
  matmul rhs has a contiguous innermost run (the stride-2 conv windows move
  to middle AP dims).  L1's input additionally splits x into even/odd column
  phases host-side so its innermost run is (xh, s) fused = 32 els.
- 16 concurrent PE subtiles per layer (tile_position 4 rows x 4 cols):
  L1: rows = 4 sample-lanes (K=18 = 6dy x 3ci), cols = 4 sample pairs.
  L2: rows = 4 lanes (K=32ci), cols = 4 blocks of 8 samples.
  L3: rows = 4 lanes, cols = 2 co-halves x 2 sample-halves.
  Col band c of each PSUM tile maps identity onto partition band c of the
  next layer's activation tensor (band = next layer's lane), so every
  PSUM->SBUF evacuation is a single 128-partition instruction.
- Evacuation: Relu(x + beta) on ACT / add+max on DVE (split), clip min(.,1)
  as separate strided DVE pass over the interior; borders memset once.
- Input DMA: 16 chunks (lane x sample-group) round-robin over the three DMA
  queues (sync, scalar, gpsimd) so the PE is fed early and HAM stays warm.
"""
import os
import sys
import numpy as np

for _p in ("/opt/trn_rl_repo", "/root/.axon_site/_ro/trn_rl_repo"):
    if os.path.isdir(_p) and _p not in sys.path:
        sys.path.append(_p)

import concourse.bass as bass
import concourse.bacc as bacc
import concourse.mybir as mybir
from concourse.tile import TileContext

EPS = 1e-5
N_CORES = 8
DT = mybir.dt.float16
F32 = mybir.dt.float32
AF = mybir.ActivationFunctionType

S = 128           # samples per core
NSG = 4           # L1 sample groups (8 samples per lane each)
SPL = 32          # samples per lane

TAPS = [(e, f) for e in range(6) for f in range(6)]


# ----------------------------------------------------------------------------
# Host-side prep
# ----------------------------------------------------------------------------

def _fold_w(w, g, b, m, v):
    """Fold conv5x5 + BN + avgpool2 into stride-2 6x6 weights + bias."""
    inv = g / np.sqrt(v + EPS)
    Wp = np.zeros((w.shape[0], w.shape[1], 6, 6), np.float32)
    for r in (0, 1):
        for s_ in (0, 1):
            Wp[:, :, r:r + 5, s_:s_ + 5] += w
    Wp *= 0.25 * inv[:, None, None, None]
    beta = (b - m * inv).astype(np.float32)
    return Wp.astype(np.float32), beta


def _lane_rep(a, groups=4):
    out = np.zeros((128, a.shape[1]), np.float32)
    for j in range(groups):
        out[32 * j:32 * j + a.shape[0]] = a
    return out


def host_prep_weights(inputs):
    W1, beta1 = _fold_w(inputs['w1'], inputs['g1'], inputs['b1'], inputs['m1'], inputs['v1'])
    W2, beta2 = _fold_w(inputs['w2'], inputs['g2'], inputs['b2'], inputs['m2'], inputs['v2'])
    W3, beta3 = _fold_w(inputs['w3'], inputs['g3'], inputs['b3'], inputs['m3'], inputs['v3'])
    inv4 = inputs['g4'] / np.sqrt(inputs['v4'] + EPS)
    beta4 = (inputs['b4'] - inputs['m4'] * inv4).astype(np.float32)
    W4 = (inputs['w4'] * inv4[:, None, None, None]).astype(np.float32)

    d = {}
    # L1 lhsT, tap t1 = xp*3+fj (dx = 2*fj+xp): [dy*3+ci, co] blocks
    wl1 = np.zeros((18, 6 * 32), np.float32)
    for xp in range(2):
        for fj in range(3):
            t1 = xp * 3 + fj
            blk = W1[:, :, :, 2 * fj + xp].transpose(2, 1, 0).reshape(18, 32)
            wl1[:, 32 * t1:32 * t1 + 32] = blk
    d['wl1'] = _lane_rep(wl1).astype(np.float16)
    # L2 lhsT per tap t=e*6+f: [32 ci, 32 co]
    d['wl2'] = _lane_rep(W2.transpose(1, 2, 3, 0).reshape(32, 36 * 32)).astype(np.float16)
    # L3 lhsT per tap: [32 ci, 64 co]
    d['wl3'] = _lane_rep(W3.transpose(1, 2, 3, 0).reshape(32, 36 * 64)).astype(np.float16)
    # L4 lhsT per tap t=u*4+v: [64 ci, 10 co], replicated into both row halves
    wl4 = W4.transpose(1, 2, 3, 0).reshape(64, 16 * 10)
    out4 = np.zeros((128, 160), np.float32)
    out4[0:64] = wl4
    out4[64:128] = wl4
    d['wl4'] = out4.astype(np.float16)

    bt = np.zeros((128, 4), np.float32)
    bt[:, 0] = np.tile(beta1, 4)
    bt[:, 1] = np.tile(beta2, 4)
    bt[:, 2] = np.tile(beta3, 2)
    bt[0:10, 3] = beta4
    d['betas'] = bt
    return d


def host_prep_x(x_core):
    """[S, 3, 32, 32] -> x_l1 [4 lanes, 18, NSG, 4608] fp16.

    x_l1[r1, dy*3+ci, sg, ((c1*16 + y)*2 + xp)*36 + xh*2 + s]
      = xpad[n, ci, 2*y + dy, 2*xh + xp],  n = 32*r1 + 8*sg + 2*c1 + s
    """
    Sc = x_core.shape[0]
    xp_ = np.zeros((Sc, 3, 36, 36), np.float32)
    xp_[:, :, 2:34, 2:34] = x_core
    # dy-expanded stride-2 rows: arr[n, dy, ci, y, x] = xpad[n, ci, 2y+dy, x]
    arr = np.stack([xp_[:, :, dy:dy + 32:2, :] for dy in range(6)], axis=1)  # [S,6,3,16,36]
    # x phase split: [S, 6, 3, 16, 2, 18] (xp, xh); then (xh, s) fused later
    arr = arr.reshape(Sc, 6, 3, 16, 18, 2).transpose(0, 1, 2, 3, 5, 4)  # [S,6,3,16,2,18]
    arr = arr.reshape(Sc, 18, 16, 2, 18)  # [n, dyci, y, xp, xh]
    # n = 32*r1 + 8*sg + 2*c1 + s
    arr = arr.reshape(4, 4, 4, 2, 18, 16, 2, 18)  # [r1, sg, c1, s, dyci, y, xp, xh]
    out = arr.transpose(0, 4, 1, 2, 5, 6, 7, 3)   # [r1, dyci, sg, c1, y, xp, xh, s]
    out = out.reshape(4, 18, NSG, 4608)
    return np.ascontiguousarray(out).astype(np.float16)


# ----------------------------------------------------------------------------
# Bass program
# ----------------------------------------------------------------------------

def build_program():
    nc = bacc.Bacc(target_bir_lowering=False)

    x_l1 = nc.dram_tensor("x_l1", [4, 18, NSG, 4608], DT, kind="ExternalInput")
    wl1 = nc.dram_tensor("wl1", [128, 192], DT, kind="ExternalInput")
    wl2 = nc.dram_tensor("wl2", [128, 1152], DT, kind="ExternalInput")
    wl3 = nc.dram_tensor("wl3", [128, 2304], DT, kind="ExternalInput")
    wl4 = nc.dram_tensor("wl4", [128, 160], DT, kind="ExternalInput")
    betas = nc.dram_tensor("betas", [128, 4], F32, kind="ExternalInput")
    y = nc.dram_tensor("y", [10, 128], F32, kind="ExternalOutput")

    with TileContext(nc) as tc:
        with tc.tile_pool(name="consts", bufs=1) as cpool:
            wl1_t = cpool.tile([128, 192], DT, name="wl1_t")
            wl2_t = cpool.tile([128, 1152], DT, name="wl2_t")
            wl3_t = cpool.tile([128, 2304], DT, name="wl3_t")
            wl4_t = cpool.tile([128, 160], DT, name="wl4_t")
            betas_t = cpool.tile([128, 4], F32, name="betas_t")
            nc.sync.dma_start(wl1_t[:, :], wl1.ap())
            nc.sync.dma_start(betas_t[:, :], betas.ap())
            deferred_w = [1]

            def flush_weights():
                if not deferred_w:
                    return
                nc.scalar.dma_start(wl2_t[:, :], wl2.ap())
                nc.scalar.dma_start(wl3_t[:, :], wl3.ap())
                nc.scalar.dma_start(wl4_t[:, :], wl4.ap())
                deferred_w.clear()

            # activations (sample-innermost)
            a2 = cpool.tile([128, 20 * 20 * 32], DT, name="a2")   # [y, x, s]
            a3 = cpool.tile([128, 12 * 12 * 32], DT, name="a3")   # [y, x, s]
            stag = cpool.tile([128, 16 * 64], DT, name="stag")    # [px, s4]
            out_sb = cpool.tile([128, 128], F32, name="out_sb")

            a2v = a2.rearrange("p (y x s) -> p y x s", y=20, x=20)
            a3v = a3.rearrange("p (y x s) -> p y x s", y=12, x=12)
            stagv = stag.rearrange("p (t s) -> p t s", t=16)

            def memset_borders(t_, W_, ns):
                # zero the pad borders (2 px) of [p, y, x, s] once
                v = t_
                row = 2 * W_ * ns          # two rows of all x, s
                nc.vector.memset(
                    bass.AP(v.tensor, v.offset,
                            [v.ap[0], [(W_ - 2) * W_ * ns, 2], [1, row]]), 0.0)
                nc.vector.memset(
                    bass.AP(v.tensor, v.offset + row,
                            [v.ap[0], [W_ * ns, W_ - 4], [(W_ - 2) * ns, 2], [1, 2 * ns]]),
                    0.0)

            memset_borders(a2, 20, 32)
            memset_borders(a3, 12, 32)

            def evac(engine_is_act, dst, src_ap, bias_ap):
                """max(x + beta, 0) on ACT or DVE."""
                if engine_is_act:
                    nc.scalar.activation(dst, src_ap, AF.Relu,
                                         bias=bias_ap, scale=1.0)
                else:
                    nc.vector.tensor_scalar(
                        dst, src_ap, bias_ap, 0.0,
                        mybir.AluOpType.add, mybir.AluOpType.max)

            dma_engines = [nc.sync, nc.scalar, nc.gpsimd]

            with (
                tc.tile_pool(name="l1io", bufs=4) as l1pool,
                tc.tile_pool(name="ps", bufs=8, space="PSUM") as pspool,
            ):
                # ================= L1 =================
                for sg in range(NSG):
                    xt = l1pool.tile([128, 4608], DT, name="xt", tag="xt")
                    xv = xt.rearrange("p (c y xp xh s) -> p c y xp xh s",
                                      c=4, y=16, xp=2, xh=18)
                    for r in range(4):
                        eng = dma_engines[(sg * 4 + r) % 3]
                        eng.dma_start(xv[32 * r:32 * r + 18],
                                      x_l1.ap()[r, :, sg])
                    if sg == 1:
                        flush_weights()
                    pl1 = [pspool.tile([128, 512], F32, name=f"ps1_{r}", tag="ps")
                           for r in range(4)]
                    for t1 in range(6):
                        xp_, fj = t1 // 3, t1 % 3
                        for r in range(4):
                            lhsT = wl1_t[32 * r:32 * r + 18, 32 * t1:32 * t1 + 32]
                            for c in range(4):
                                rhs = xv[32 * r:32 * r + 18, c, :, xp_,
                                         fj:fj + 16, :]
                                nc.tensor.matmul(
                                    pl1[r][32 * c:32 * c + 32, :], lhsT, rhs,
                                    start=(t1 == 0), stop=(t1 == 5),
                                    skip_group_check=True,
                                    tile_position=(32 * r, 32 * c),
                                )
                    # evac: relu(x + beta1); dst slots 8sg+2r+{0,1}
                    for r in range(4):
                        src = pl1[r][:, :].rearrange(
                            "p (y x s) -> p y x s", y=16, x=16)
                        sl = 8 * sg + 2 * r
                        dst = a2v[:, 2:18, 2:18, sl:sl + 2]
                        evac(r % 2 == 0, dst, src, betas_t[:, 0:1])
                    # upper clip for this sg's 8 slots (interior only)
                    nc.vector.tensor_scalar_min(
                        a2v[:, 2:18, 2:18, 8 * sg:8 * sg + 8],
                        a2v[:, 2:18, 2:18, 8 * sg:8 * sg + 8], 1.0)

                # ================= L2 =================
                pl2 = [pspool.tile([128, 512], F32, name=f"ps2_{r}", tag="ps")
                       for r in range(4)]
                for t, (e, f) in enumerate(TAPS):
                    for r2 in range(4):
                        lhsT = wl2_t[32 * r2:32 * r2 + 32, 32 * t:32 * t + 32]
                        for c2 in range(4):
                            rhs = a2v[32 * r2:32 * r2 + 32, e:e + 15:2,
                                      f:f + 15:2, 8 * c2:8 * c2 + 8]
                            nc.tensor.matmul(
                                pl2[r2][32 * c2:32 * c2 + 32, :], lhsT, rhs,
                                start=(t == 0), stop=(t == 35),
                                skip_group_check=True,
                                tile_position=(32 * r2, 32 * c2),
                            )
                for r2 in range(4):
                    src = pl2[r2][:, :].rearrange(
                        "p (y x s) -> p y x s", y=8, x=8)
                    dst = a3v[:, 2:10, 2:10, 8 * r2:8 * r2 + 8]
                    evac(r2 % 2 == 0, dst, src, betas_t[:, 1:2])
                for h in range(2):
                    nc.vector.tensor_scalar_min(
                        a3v[:, 2:10, 2:10, 16 * h:16 * h + 16],
                        a3v[:, 2:10, 2:10, 16 * h:16 * h + 16], 1.0)

                # ================= L3 =================
                pl3 = [pspool.tile([128, 256], F32, name=f"ps3_{r}", tag="ps")
                       for r in range(4)]
                for t, (e, f) in enumerate(TAPS):
                    for r3 in range(4):
                        for cg in range(2):
                            lhsT = wl3_t[32 * r3:32 * r3 + 32,
                                         64 * t + 32 * cg:64 * t + 32 * cg + 32]
                            for sh in range(2):
                                band = 2 * sh + cg
                                rhs = a3v[32 * r3:32 * r3 + 32, e:e + 7:2,
                                          f:f + 7:2, 16 * sh:16 * sh + 16]
                                nc.tensor.matmul(
                                    pl3[r3][32 * band:32 * band + 32, :], lhsT, rhs,
                                    start=(t == 0), stop=(t == 35),
                                    skip_group_check=True,
                                    tile_position=(32 * r3, 32 * band),
                                )
                for r3 in range(4):
                    src = pl3[r3][:, :].rearrange(
                        "p (y x s) -> p (y x) s", y=4, x=4)
                    dst = stagv[:, :, 16 * r3:16 * r3 + 16]
                    evac(r3 % 2 == 0, dst, src, betas_t[:, 2:3])
                nc.vector.tensor_scalar_min(stag[:, :], stag[:, :], 1.0)

                # ================= L4 =================
                ps4 = [pspool.tile([128, 64], F32, name=f"ps4_{k}", tag="ps")
                       for k in range(2)]
                for t in range(16):
                    for sh in range(2):
                        lhsT = wl4_t[64 * sh:64 * sh + 64, 10 * t:10 * t + 10]
                        rhs = stagv[64 * sh:64 * sh + 64, t, :]
                        nc.tensor.matmul(
                            ps4[sh][0:10, :], lhsT, rhs,
                            start=(t == 0), stop=(t == 15),
                            skip_group_check=True,
                            tile_position=(64 * sh, 0),
                        )
                for sh in range(2):
                    nc.scalar.activation(
                        out_sb[0:10, 64 * sh:64 * sh + 64], ps4[sh][0:10, :],
                        AF.Identity, bias=betas_t[0:10, 3:4], scale=1.0,
                    )
                nc.sync.dma_start(y.ap(), out_sb[0:10, :])

        return nc


_NC_CACHE = None


def get_program():
    global _NC_CACHE
    if _NC_CACHE is None:
        nc = build_program()
        if not nc.is_finalized():
            nc.finalize()
        _NC_CACHE = nc
    return _NC_CACHE


def make_in_maps(inputs, n_cores=N_CORES):
    wdict = host_prep_weights(inputs)
    in_maps = []
    for c in range(n_cores):
        x_core = np.asarray(inputs['x_in'][c * S:(c + 1) * S], np.float32)
        m = {'x_l1': host_prep_x(x_core)}
        m.update(wdict)
        in_maps.append(m)
    return in_maps


def _core_sample(col):
    """Output column (0..127) -> per-core sample index n."""
    sh, s4 = col // 64, col % 64
    r3, k = s4 // 16, s4 % 16
    slot3 = 16 * sh + k
    rem3 = slot3 % 8
    # n = 32*r1 + 8*sg + 2*c1 + s1
    return 32 * (rem3 // 2) + 8 * r3 + 2 * (slot3 // 8) + rem3 % 2


def assemble_output(results, n_cores=N_CORES):
    out = np.zeros((n_cores * S, 10), np.float32)
    cols = np.array([_core_sample(c) for c in range(S)])
    for c in range(n_cores):
        yc = np.asarray(results[c]['y'])  # [10, 128]
        out[c * S + cols, :] = yc.T
    return out


def kernel(**inputs) -> np.ndarray:
    from concourse.bass_utils import run_bass_kernel_spmd
    nc = get_program()
    in_maps = make_in_maps(inputs)
    res = run_bass_kernel_spmd(nc, in_maps, list(range(N_CORES)))
    return assemble_output(res.results)


# revision 8
# speedup vs baseline: 1.9112x; 1.7310x over previous
"""Trainium2 Bass kernel for nn_CONV_tiny_add_partial_558345748883.

Network: 3x [conv5x5(pad2) -> BN -> avgpool2 -> clip01] -> conv4x4(valid) -> BN1d
Input x_in [1024, 3, 32, 32] f32; output [1024, 10] f32.

v3 strategy
-----------
Measured law: per-MATMUL cost ~34ns (sem-completion serialization) once >=7
subtiles are concurrent; per-tile stream 1 col/cycle; so minimize MM count by
maximizing N*M per MM (N<=512, M = output width).

- K-packing via phase-split activation layouts (zero-copy):
  L1: x columns phase-split host-side -> K=36 (2 dx-phases x 6dy x 3ci),
      3 taps, 8 tiles (2 row-groups x 4 col bands), M=32, N=512.
  a2/a3 stored Y-PARITY-SPLIT: partitions [lane(2) x parity(2) x ci(32)].
  The L1/L2 MMs write psum col bands = (dst-lane, out-y-parity), so the
  PSUM->SBUF evac is identity on all 128 partitions, and the next layer
  gets K=64 = (parity, ci) with tap PAIRS at a uniform free-dim offset:
  L2: 18 passes (6 dx x 3 dy-pairs), 8 tiles 64x32, N=512.
  L3: 18 passes, 4 tiles 64x64 (M=64 native), N=512.
  L4: stag [parity x 64ci]; 16 taps x 1 MM (N=128), rows alternate.
- fp8e4 for L1 input + wl1 (halves input DMA; rel err ~1.1e-2 < 2e-2).
- Input DMA: 8 chunks on sync/scalar queues only, sg0+1 first, weights after.
- Evac: Relu(x+beta) split ACT/DVE; upper clip min(.,1) strided DVE passes.
"""
import os
import sys
import numpy as np

for _p in ("/opt/trn_rl_repo", "/root/.axon_site/_ro/trn_rl_repo"):
    if os.path.isdir(_p) and _p not in sys.path:
        sys.path.append(_p)

import concourse.bass as bass
import concourse.bacc as bacc
import concourse.mybir as mybir
from concourse.tile import TileContext

EPS = 1e-5
N_CORES = 8
DT = mybir.dt.float16
FP8 = mybir.dt.float8e4
F32 = mybir.dt.float32
AF = mybir.ActivationFunctionType

S = 128


# ----------------------------------------------------------------------------
# Host-side prep
# ----------------------------------------------------------------------------

def _fold_w(w, g, b, m, v):
    inv = g / np.sqrt(v + EPS)
    Wp = np.zeros((w.shape[0], w.shape[1], 6, 6), np.float32)
    for r in (0, 1):
        for s_ in (0, 1):
            Wp[:, :, r:r + 5, s_:s_ + 5] += w
    Wp *= 0.25 * inv[:, None, None, None]
    beta = (b - m * inv).astype(np.float32)
    return Wp.astype(np.float32), beta


def host_prep_weights(inputs):
    W1, beta1 = _fold_w(inputs['w1'], inputs['g1'], inputs['b1'], inputs['m1'], inputs['v1'])
    W2, beta2 = _fold_w(inputs['w2'], inputs['g2'], inputs['b2'], inputs['m2'], inputs['v2'])
    W3, beta3 = _fold_w(inputs['w3'], inputs['g3'], inputs['b3'], inputs['m3'], inputs['v3'])
    inv4 = inputs['g4'] / np.sqrt(inputs['v4'] + EPS)
    beta4 = (inputs['b4'] - inputs['m4'] * inv4).astype(np.float32)
    W4 = (inputs['w4'] * inv4[:, None, None, None]).astype(np.float32)

    d = {}
    # L1 lhsT per tap fj (dx = 2*fj + xp): K=36 rows (xp, dy, ci), M=32 co.
    wl1 = np.zeros((36, 3 * 32), np.float32)
    for fj in range(3):
        for xp in range(2):
            blk = W1[:, :, :, 2 * fj + xp].transpose(2, 1, 0).reshape(18, 32)
            wl1[18 * xp:18 * xp + 18, 32 * fj:32 * fj + 32] = blk
    w1r = np.zeros((128, 96), np.float32)
    w1r[0:36] = wl1
    w1r[64:100] = wl1
    d['wl1'] = w1r.astype(mybir.dt.np(FP8))
    # L2 lhsT per pass t2 = j*6+f (dy pair e = 2j+q): K=64 rows (q, ci), M=32.
    wl2 = np.zeros((64, 18 * 32), np.float32)
    for j in range(3):
        for f in range(6):
            t2 = j * 6 + f
            for q in range(2):
                wl2[32 * q:32 * q + 32, 32 * t2:32 * t2 + 32] = \
                    W2[:, :, 2 * j + q, f].T
    w2r = np.zeros((128, 576), np.float32)
    w2r[0:64] = wl2
    w2r[64:128] = wl2
    d['wl2'] = w2r.astype(np.float16)
    # L3 lhsT per pass: K=64 (q, ci), M=64 co.
    wl3 = np.zeros((64, 18 * 64), np.float32)
    for j in range(3):
        for f in range(6):
            t2 = j * 6 + f
            for q in range(2):
                wl3[32 * q:32 * q + 32, 64 * t2:64 * t2 + 64] = \
                    W3[:, :, 2 * j + q, f].T
    w3r = np.zeros((128, 1152), np.float32)
    w3r[0:64] = wl3
    w3r[64:128] = wl3
    d['wl3'] = w3r.astype(np.float16)
    # L4 lhsT per tap t = u*4+v: K=64 ci, M=10; row half = u parity.
    wl4 = W4.transpose(1, 2, 3, 0).reshape(64, 16 * 10)
    w4r = np.zeros((128, 160), np.float32)
    w4r[0:64] = wl4
    w4r[64:128] = wl4
    d['wl4'] = w4r.astype(np.float16)

    bt = np.zeros((128, 4), np.float32)
    bt[:, 0] = np.tile(beta1, 4)
    bt[:, 1] = np.tile(beta2, 4)
    bt[:, 2] = np.tile(beta3, 2)
    bt[0:10, 3] = beta4
    d['betas'] = bt
    return d


def host_prep_x(x_core):
    """[128, 3, 32, 32] -> x_l1 [2 R, 36, 4 cp, 4608] fp8.

    Partition k = 18*xp + 3*dy + ci (K=36).  Free, per chunk-pair cp
    (2 sgs): [sg2, j2, y16, xh18, sf4].
    x_l1[R, k, cp, ...] = xpad[n, ci, 2y+dy, 2xh+xp],
      n = ((sg*2 + R)*2 + j)*4 + sf,  sg = 2*cp + sg2.
    """
    xp_ = np.zeros((128, 3, 36, 36), np.float32)
    xp_[:, :, 2:34, 2:34] = x_core
    # [n, dy, ci, y, x] stride-2 rows
    arr = np.stack([xp_[:, :, dy:dy + 32:2, :] for dy in range(6)], axis=1)
    # x phase split -> [n, xp, dy, ci, y, xh]
    arr = arr.reshape(128, 6, 3, 16, 18, 2).transpose(0, 5, 1, 2, 3, 4)
    arr = arr.reshape(128, 36, 16, 18)          # [n, k, y, xh]
    # n = ((sg*2+R)*2+j)*4+sf -> [sg8, R2, j2, sf4]
    arr = arr.reshape(8, 2, 2, 4, 36, 16, 18)   # [sg, R, j, sf, k, y, xh]
    arr = arr.reshape(4, 2, 2, 2, 4, 36, 16, 18)  # [cp, sg2, R, j, sf, k, y, xh]
    out = arr.transpose(2, 5, 0, 1, 3, 6, 7, 4)   # [R, k, cp, sg2, j, y, xh, sf]
    out = out.reshape(2, 36, 4, 4608)
    return np.ascontiguousarray(out).astype(mybir.dt.np(FP8))


# ----------------------------------------------------------------------------
# Bass program
# ----------------------------------------------------------------------------

def build_program():
    nc = bacc.Bacc(target_bir_lowering=False)

    x_l1 = nc.dram_tensor("x_l1", [2, 36, 4, 4608], FP8, kind="ExternalInput")
    wl1 = nc.dram_tensor("wl1", [128, 96], FP8, kind="ExternalInput")
    wl2 = nc.dram_tensor("wl2", [128, 576], DT, kind="ExternalInput")
    wl3 = nc.dram_tensor("wl3", [128, 1152], DT, kind="ExternalInput")
    wl4 = nc.dram_tensor("wl4", [128, 160], DT, kind="ExternalInput")
    betas = nc.dram_tensor("betas", [128, 4], F32, kind="ExternalInput")
    y = nc.dram_tensor("y", [10, 128], F32, kind="ExternalOutput")

    with TileContext(nc) as tc:
        with tc.tile_pool(name="consts", bufs=1) as cpool:
            wl1_t = cpool.tile([128, 96], FP8, name="wl1_t")
            wl2_t = cpool.tile([128, 576], DT, name="wl2_t")
            wl3_t = cpool.tile([128, 1152], DT, name="wl3_t")
            wl4_t = cpool.tile([128, 160], DT, name="wl4_t")
            betas_t = cpool.tile([128, 4], F32, name="betas_t")
            nc.sync.dma_start(wl1_t[:, :], wl1.ap())
            nc.sync.dma_start(betas_t[:, :], betas.ap())
            deferred_w = [1]

            def flush_weights():
                if not deferred_w:
                    return
                nc.scalar.dma_start(wl2_t[:, :], wl2.ap())
                nc.scalar.dma_start(wl3_t[:, :], wl3.ap())
                nc.scalar.dma_start(wl4_t[:, :], wl4.ap())
                deferred_w.clear()

            # a2: [lane2 x par2 x ci32][yh10, x20, s64]; a3: [yh6, x12, s64]
            a2 = cpool.tile([128, 10 * 20 * 64], DT, name="a2")
            a3 = cpool.tile([128, 6 * 12 * 64], DT, name="a3")
            stag = cpool.tile([128, 8 * 128], DT, name="stag")  # [par x ci][px8, s128]
            out_sb = cpool.tile([128, 128], F32, name="out_sb")

            a2v = a2.rearrange("p (y x s) -> p y x s", y=10, x=20)
            a3v = a3.rearrange("p (y x s) -> p y x s", y=6, x=12)
            stagv = stag.rearrange("p (t s) -> p t s", t=8)

            def memset_borders(t_, YH, W_, ns):
                # phase-plane pad: first+last yh row, and 1 col each side
                v = t_[:, :]
                nc.vector.memset(
                    bass.AP(v.tensor, v.offset,
                            [v.ap[0], [(YH - 1) * W_ * ns, 2], [1, W_ * ns]]), 0.0)
                nc.vector.memset(
                    bass.AP(v.tensor, v.offset + W_ * ns,
                            [v.ap[0], [W_ * ns, YH - 2], [(W_ - 2) * ns, 2], [1, 2 * ns]]),
                    0.0)

            memset_borders(a2, 10, 20, 64)
            memset_borders(a3, 6, 12, 64)

            def evac(engine_is_act, dst, src_ap, bias_ap):
                if engine_is_act:
                    nc.scalar.activation(dst, src_ap, AF.Relu,
                                         bias=bias_ap, scale=1.0)
                else:
                    nc.vector.tensor_scalar(
                        dst, src_ap, bias_ap, 0.0,
                        mybir.AluOpType.add, mybir.AluOpType.max)

            with (
                tc.tile_pool(name="l1io", bufs=3) as l1pool,
                tc.tile_pool(name="ps", bufs=8, space="PSUM") as pspool,
            ):
                # ================= L1 =================
                # chunk-pair cp covers sgs {2cp, 2cp+1}
                for cp in range(4):
                    xt = l1pool.tile([128, 4608], FP8, name="xt", tag="xt")
                    xv = xt.rearrange("p (g j y xh s) -> p g j y xh s",
                                      g=2, j=2, y=16, xh=18)
                    for R in range(2):
                        eng = nc.sync if (cp + R) % 2 == 0 else nc.scalar
                        eng.dma_start(xv[64 * R:64 * R + 36],
                                      x_l1.ap()[R, :, cp])
                    if cp == 1:
                        flush_weights()
                    for g in range(2):
                        sg = 2 * cp + g
                        pl1 = [pspool.tile([128, 512], F32, name=f"ps1_{R}", tag="ps")
                               for R in range(2)]
                        for fj in range(3):
                            for R in range(2):
                                lhsT = wl1_t[64 * R:64 * R + 36,
                                             32 * fj:32 * fj + 32]
                                for j in range(2):
                                    for p in range(2):
                                        rhs = xv[64 * R:64 * R + 36, g, j,
                                                 p:16:2, fj:fj + 16, :]
                                        nc.tensor.matmul(
                                            pl1[R][64 * j + 32 * p:64 * j + 32 * p + 32, :],
                                            lhsT, rhs,
                                            start=(fj == 0), stop=(fj == 2),
                                            skip_group_check=True,
                                            tile_position=(64 * R, 64 * j + 32 * p),
                                        )
                        # evac: psum [128 = (j,p) x 32co][y8, x16, sf4]
                        # dst a2 plane-p rows yh = (2k+p+2)//2 = k+1 for k=0..7
                        for R in range(2):
                            src = pl1[R][:, :].rearrange(
                                "p (y x s) -> p y x s", y=8, x=16)
                            sl = 8 * sg + 4 * R
                            dst = a2v[:, 1:9, 2:18, sl:sl + 4]
                            evac(R == 0, dst, src, betas_t[:, 0:1])
                        nc.vector.tensor_scalar_min(
                            a2v[:, 1:9, 2:18, 8 * sg:8 * sg + 8],
                            a2v[:, 1:9, 2:18, 8 * sg:8 * sg + 8], 1.0)

                # ================= L2 =================
                # passes t2 = j*6+f: K=64 (parity q, ci); out-y parity p' banded
                pl2 = [pspool.tile([128, 512], F32, name=f"ps2_{k}", tag="ps")
                       for k in range(4)]  # k = 2*L + r2
                for t2 in range(18):
                    j, f = t2 // 6, t2 % 6
                    for L in range(2):
                        lhsT = wl2_t[64 * L:64 * L + 64, 32 * t2:32 * t2 + 32]
                        for r2 in range(2):
                            for jp in range(2):  # dst lane j'
                                for p in range(2):  # out-y parity
                                    rhs = a2v[64 * L:64 * L + 64,
                                              j + p:j + p + 7:2, f:f + 15:2,
                                              32 * r2 + 16 * jp:32 * r2 + 16 * jp + 16]
                                    nc.tensor.matmul(
                                        pl2[2 * L + r2][64 * jp + 32 * p:
                                                        64 * jp + 32 * p + 32, :],
                                        lhsT, rhs,
                                        start=(t2 == 0), stop=(t2 == 17),
                                        skip_group_check=True,
                                        tile_position=(64 * L, 64 * jp + 32 * p),
                                    )
                for k in range(4):
                    L, r2 = k // 2, k % 2
                    src = pl2[k][:, :].rearrange(
                        "p (y x s) -> p y x s", y=4, x=8)
                    sl = 16 * (2 * L + r2)
                    dst = a3v[:, 1:5, 2:10, sl:sl + 16]
                    evac(k % 2 == 0, dst, src, betas_t[:, 1:2])
                    nc.vector.tensor_scalar_min(
                        a3v[:, 1:5, 2:10, sl:sl + 16],
                        a3v[:, 1:5, 2:10, sl:sl + 16], 1.0)

                # ================= L3 =================
                # 4 tiles: rows = a3 lane L', cols = out-y parity band p''
                pl3 = [pspool.tile([128, 512], F32, name=f"ps3_{k}", tag="ps")
                       for k in range(2)]  # k = L'
                for t2 in range(18):
                    j, f = t2 // 6, t2 % 6
                    for Lp in range(2):
                        lhsT = wl3_t[64 * Lp:64 * Lp + 64, 64 * t2:64 * t2 + 64]
                        for p in range(2):
                            rhs = a3v[64 * Lp:64 * Lp + 64,
                                      j + p:j + p + 3:2, f:f + 7:2, :]
                            nc.tensor.matmul(
                                pl3[Lp][64 * p:64 * p + 64, :], lhsT, rhs,
                                start=(t2 == 0), stop=(t2 == 17),
                                skip_group_check=True,
                                tile_position=(64 * Lp, 64 * p),
                            )
                # evac: psum [128 = par x 64co][y2, x4, s64] -> stag px = y*4+x
                for Lp in range(2):
                    src = pl3[Lp][:, :].rearrange(
                        "p (y x s) -> p (y x) s", y=2, x=4)
                    dst = stagv[:, :, 64 * Lp:64 * Lp + 64]
                    evac(Lp == 0, dst, src, betas_t[:, 2:3])
                nc.vector.tensor_scalar_min(stag[:, :], stag[:, :], 1.0)

                # ================= L4 =================
                # separate psum per row-parity stream (avoid concurrent
                # accumulation races into one region), then add at evac
                ps4 = [pspool.tile([128, 128], F32, name=f"ps4_{q}", tag="ps")
                       for q in range(2)]
                nseen = [0, 0]
                for t in range(16):
                    u, v = t // 4, t % 4
                    q = u % 2
                    lhsT = wl4_t[64 * q:64 * q + 64, 10 * t:10 * t + 10]
                    rhs = stagv[64 * q:64 * q + 64, (u // 2) * 4 + v, :]
                    nc.tensor.matmul(
                        ps4[q][0:10, :], lhsT, rhs,
                        start=(nseen[q] == 0), stop=(nseen[q] == 7),
                        skip_group_check=True,
                        tile_position=(64 * q, 0),
                    )
                    nseen[q] += 1
                nc.scalar.activation(
                    out_sb[0:10, :], ps4[0][0:10, :],
                    AF.Identity, bias=betas_t[0:10, 3:4], scale=1.0,
                )
                nc.vector.tensor_tensor(
                    out_sb[0:10, :], ps4[1][0:10, :], out_sb[0:10, :],
                    mybir.AluOpType.add)
                nc.sync.dma_start(y.ap(), out_sb[0:10, :])

        return nc


_NC_CACHE = None


def get_program():
    global _NC_CACHE
    if _NC_CACHE is None:
        nc = build_program()
        if not nc.is_finalized():
            nc.finalize()
        _NC_CACHE = nc
    return _NC_CACHE


def make_in_maps(inputs, n_cores=N_CORES):
    wdict = host_prep_weights(inputs)
    in_maps = []
    for c in range(n_cores):
        x_core = np.asarray(inputs['x_in'][c * S:(c + 1) * S], np.float32)
        m = {'x_l1': host_prep_x(x_core)}
        m.update(wdict)
        in_maps.append(m)
    return in_maps


def _core_sample(col):
    """Output column (0..127) -> per-core sample index n."""
    Lp, t = col // 64, col % 64          # stag: s3 = 64*L' + slot2
    half, k = t // 16, t % 16            # slot2 = 16*(2L + r2) + k
    L, r2 = half // 2, half % 2
    jp = Lp                              # dst a3 lane = j'
    s1 = 32 * r2 + 16 * jp + k           # a2 slot of lane L
    sg, rem = s1 // 8, s1 % 8
    R, sf = rem // 4, rem % 4
    return ((sg * 2 + R) * 2 + L) * 4 + sf


def assemble_output(results, n_cores=N_CORES):
    out = np.zeros((n_cores * S, 10), np.float32)
    cols = np.array([_core_sample(c) for c in range(S)])
    for c in range(n_cores):
        yc = np.asarray(results[c]['y'])
        out[c * S + cols, :] = yc.T
    return out


def kernel(**inputs) -> np.ndarray:
    from concourse.bass_utils import run_bass_kernel_spmd
    nc = get_program()
    in_maps = make_in_maps(inputs)
    res = run_bass_kernel_spmd(nc, in_maps, list(range(N_CORES)))
    return assemble_output(res.results)
